# revision 1
# baseline (speedup 1.0000x reference)
"""Dense 2-layer GAT on 8 Trainium2 NeuronCores (Bass/Tile, SPMD) — v2.

Problem: B=4 graphs, N=2048 nodes, F_in=128, H=8 heads, F_hid=64, C=32.
Sharding: 2 cores per graph, each core owns 1024 attention rows (r-shard)
for all heads in layer 1 and for layer 2; only a tiny [1024,65] AllGather
of Wh2 crosses cores between the layers.

v2 rebalance (guided by the CoreSim cost model, validated on HW):
  - DVE perf modes: tensor_scalar (single tensor input, immediate or
    per-partition ptr scalar) runs 4x on packed f16; tensor_tensor runs
    2x; scalar_tensor_tensor runs 1x. The old kernel built everything
    from stt (1x).
  - ACT table sets: Exp, Ln, Copy and parametric_relu (Prelu) share one
    table -> leaky-relu can run on ACT with a fused per-partition f2
    bias and zero table reloads.
  - GPSIMD (Pool) HW limits: only TensorScalar with immediate scalars,
    Memset and TensorCopy are legal (no TensorTensor, no ptr scalars,
    no PSUM access) -- it contributes the alpha-mult of the lrelu.
  - Per (head-pair, chunk) iteration, scheme P:
        u[:,h] = ts(madj[cb] + f2_ptr)        DVE 4x
        u     += f1pair[hp]                   DVE tt 2x
        au     = ts(u * alpha)                Pool (ts-imm)
        u      = max(u, au)                   DVE tt 2x
        p      = exp(u)                       ACT
    scheme A (1/3 of L1 sites, uses ACT slack to unload DVE):
        u[:,h] = Prelu(f1rep_h + f2_bias)     ACT (fused lrelu+outer-add)
        u     += madj[cb] (bcast both heads)  DVE tt 2x
        p      = exp(u)                       ACT
    Mask stays additive (-150) and flows through lrelu either way:
    non-edges end below -29 pre-exp -> exp underflows f16 to 0.
  - Softmax normalizer via DVE reciprocal (not ACT Ln/Exp); epilogue
    min/add on Pool; last head-pair epilogue and the final output
    epilogue are split into row-halves to shorten the serial tails.
  - madj[cc] += g1 runs inside the AllGather window so post-collective
    layer-2 iterations only need a 4x ts-add of the g2 scalar.
  - The inter-layer AllGather ships only the 33 live columns (32 Wh2 +
    g2); the 32 ones-columns are memset locally during the collective.
  - Setup matmul operands (xT/wall/wa2/w1rep/xtr) ship as f16:
    4x faster PE prologue (853ns -> 213ns per Wh matmul).
  Simulated core time 269us vs 426us for the all-stt baseline.
"""
import os
import numpy as np
import ml_dtypes

import concourse.bass as bass
import concourse.mybir as mybir
import concourse.tile as tile
from concourse.bass_utils import run_bass_kernel_spmd
from concourse.vector_clock import VectorClock, ScopedClock

F32 = mybir.dt.float32
F16 = mybir.dt.float16
Alu = mybir.AluOpType
Act = mybir.ActivationFunctionType

B, N, F_IN, H, F_HID, C = 4, 2048, 128, 8, 64, 32
RSH = N // 2           # rows per core
NCB = N // 128         # 16 c-chunks
ALPHA = 0.2
OUT_SLOPE = 0.01
MASK = -150.0
GROUPS = [[0, 1], [2, 3], [4, 5], [6, 7]]

# Per-site scheme for the 72 (head-pair, chunk) iterations (64 L1 + 8 L2):
#   P = DVE-heavy (ts/tt + Pool max), A = ACT-heavy (Prelu+bias).
# Default: A every 7th site (~14%) to balance DVE vs ACT.
_DEF_SCHED = ("".join("A" if i % 3 == 2 else "P" for i in range(64))
              + "P" * 8)  # L2 tail is ACT-paced: keep it off ACT
SCHED = os.environ.get("GAT_SCHED", _DEF_SCHED)

# ---------------------------------------------------------------------------
# Patch: Tile's kernel-tail drain aggregates one wait per outstanding proc
# lane into a single Drain instruction; walrus codegen only supports one
# sync wait per instruction ("Too many sync wait commands").  Split into a
# chain of single-wait drains instead.
import concourse.tile as _tile_mod

_ORIG_DRAIN = _tile_mod.TileContext._drain_and_barrier


def _split_drain_and_barrier(self, tick_clock, wait_clock):
    vals = eval(repr(tick_clock.global_clock).split("VectorClock(", 1)[1].rstrip(")"))
    for i, v in enumerate(vals):
        if v <= 0:
            continue
        part = VectorClock()
        part.require_at_least(i, v)
        d = self.nc.sync.drain()
        wait_clock.add_sem_waits(d.ins, ScopedClock({None: part}))
    self.nc.sync.drain()
    self.nc.all_engine_barrier()
    popped = self.nc._tile_sem_poison_stack.pop()
    assert popped is self._sem_poison
    self.nc.clear_and_free_semaphores(list(self.sems.allocated().values()))
    self.nc.all_engine_barrier()


_tile_mod.TileContext._drain_and_barrier = _split_drain_and_barrier


def _legalize_multi_waits(nc):
    """Walrus codegen accepts at most one sync wait per instruction; hoist
    extra waits onto preceding same-engine sequencer NOPs."""
    Op = nc.isa.Opcode

    def mk_nop(engine):
        return nc.engines[engine]._isa(Op.NEURON_ISA_TPB_OPCODE_NOP, {})

    n_fix = 0
    for f in nc.m.functions:
        for bb in f.blocks:
            insts = list(bb.instructions)
            if not any(i.sync_info and i.sync_info.on_wait
                       and len(i.sync_info.on_wait) > 1 for i in insts):
                continue
            new = []
            for inst in insts:
                si = inst.sync_info
                if si and si.on_wait and len(si.on_wait) > 1:
                    waits = list(si.on_wait)
                    for w in waits[:-1]:
                        nop = mk_nop(inst.engine)
                        nop.sync_info = mybir.SyncInfo(on_wait=[w], on_update=[])
                        new.append(nop)
                        n_fix += 1
                    inst.sync_info = mybir.SyncInfo(
                        on_wait=[waits[-1]], on_update=list(si.on_update or []))
                new.append(inst)
            bb.instructions = new
    return n_fix
# ---------------------------------------------------------------------------


def build_nc(legalize=True):
    nc = bass.Bass(num_devices=8)

    xT_e = nc.dram_tensor("xT", [F_IN, N], F16, kind="ExternalInput")
    xtr_e = nc.dram_tensor("xtr", [F_IN, RSH], F16, kind="ExternalInput")
    madj_e = nc.dram_tensor("madj", [N, RSH], F16, kind="ExternalInput")
    wall_e = nc.dram_tensor("wall", [F_IN, H * F_HID], F16, kind="ExternalInput")
    wa2_e = nc.dram_tensor("wa2", [F_IN, H], F16, kind="ExternalInput")
    w1rep_e = nc.dram_tensor("w1rep", [F_IN, H * 128], F16, kind="ExternalInput")
    woaug_e = nc.dram_tensor("woaug", [H * F_HID, 65], F16, kind="ExternalInput")
    wo1rep_e = nc.dram_tensor("wo1rep", [H * F_HID, 128], F16, kind="ExternalInput")
    out_e = nc.dram_tensor("out", [C, RSH], F32, kind="ExternalOutput")
    cc_in = nc.dram_tensor("cc_in", [RSH, 33], F16)
    cc_out = nc.dram_tensor("cc_out", [N, 33], F16)

    with tile.TileContext(nc) as tc:
        from contextlib import ExitStack
        with ExitStack() as ctx:
            res = ctx.enter_context(tc.tile_pool(name="res", bufs=1))
            work = ctx.enter_context(tc.tile_pool(name="work", bufs=4))
            ep = ctx.enter_context(tc.tile_pool(name="ep", bufs=2))
            fin = ctx.enter_context(tc.tile_pool(name="fin", bufs=1))

            # ---------------- input loads ----------------
            # order: what the first Wh matmul + first L1 iteration need first
            wall = res.tile([F_IN, H * F_HID], F16, tag="wall")
            nc.sync.dma_start(out=wall, in_=wall_e[:, :])
            xT = res.tile([F_IN, N], F16, tag="xT")
            nc.sync.dma_start(out=xT[:, 0:512], in_=xT_e[:, 0:512])
            wa2 = res.tile([F_IN, H], F16, tag="wa2")
            nc.sync.dma_start(out=wa2, in_=wa2_e[:, :])
            xtr = res.tile([F_IN, RSH], F16, tag="xtr")
            nc.sync.dma_start(out=xtr, in_=xtr_e[:, :])
            w1rep = res.tile([F_IN, H * 128], F16, tag="w1rep")
            nc.sync.dma_start(out=w1rep, in_=w1rep_e[:, :])
            madj = [res.tile([128, RSH], F16, tag=f"madj{cb}", name=f"madj{cb}")
                    for cb in range(NCB)]
            for cb in range(2):
                nc.sync.dma_start(out=madj[cb], in_=madj_e[cb * 128:(cb + 1) * 128, :])
            for j in range(1, 4):
                nc.sync.dma_start(out=xT[:, j * 512:(j + 1) * 512],
                                  in_=xT_e[:, j * 512:(j + 1) * 512])
            for cb in range(2, NCB):
                nc.sync.dma_start(out=madj[cb], in_=madj_e[cb * 128:(cb + 1) * 128, :])
            woaug = [res.tile([128, 65], F16, tag=f"woaug{k}", name=f"woaug{k}") for k in range(4)]
            wo1rep = [res.tile([128, 128], F16, tag=f"wo1rep{k}", name=f"wo1rep{k}") for k in range(4)]
            for k in range(4):
                nc.sync.dma_start(out=woaug[k], in_=woaug_e[k * 128:(k + 1) * 128, :])
                nc.sync.dma_start(out=wo1rep[k], in_=wo1rep_e[k * 128:(k + 1) * 128, :])

            whaug = [res.tile([128, H * 128], F16, tag=f"whaug{cb}", name=f"whaug{cb}") for cb in range(NCB)]
            # f1 for a head PAIR side by side: [128, 2*RSH]
            f1pair = [res.tile([128, 2 * RSH], F16, tag=f"f1pair{hp}", name=f"f1pair{hp}")
                      for hp in range(H // 2)]
            f2sb = res.tile([128, NCB * H], F32, tag="f2sb")
            hT = [res.tile([128, RSH], F16, tag=f"hT{k}", name=f"hT{k}") for k in range(4)]

            with tc.tile_pool(name="ps_set", bufs=2, space="PSUM") as ps_set:
                # f1 for the first head pair before anything else: the first
                # L1 iteration needs f1pair[0]; the Wh chunk loop is long.
                def emit_f1(h):
                    pf1 = ps_set.tile([128, RSH], F32, tag="set_f1")
                    for j in range(2):
                        nc.tensor.matmul(pf1[:, j * 512:(j + 1) * 512],
                                         lhsT=w1rep[:, h * 128:(h + 1) * 128],
                                         rhs=xtr[:, j * 512:(j + 1) * 512],
                                         start=True, stop=True)
                    dst = f1pair[h // 2][:, (h % 2) * RSH:(h % 2 + 1) * RSH]
                    if h >= 2 and h % 2 == 0:
                        nc.vector.tensor_copy(out=dst, in_=pf1)
                    else:
                        nc.scalar.activation(out=dst, in_=pf1, func=Act.Copy)
                for h in (0, 1):
                    emit_f1(h)
                # Wh per c-chunk: [128, 512] = all heads side by side
                for cb in range(NCB):
                    pwh = ps_set.tile([128, H * F_HID], F32, tag="set_a")
                    nc.tensor.matmul(pwh, lhsT=xT[:, cb * 128:(cb + 1) * 128],
                                     rhs=wall, start=True, stop=True)
                    # strided copy into whaug (64 Wh cols of each 128-col head block)
                    wh_v = whaug[cb].rearrange("p (hh q) -> p hh q", q=128)
                    dst = wh_v[:, :, 0:F_HID]
                    src = pwh.rearrange("p (hh o) -> p hh o", o=F_HID)
                    if cb % 2 == 0:
                        nc.vector.tensor_copy(out=dst, in_=src)
                    else:
                        nc.scalar.activation(out=dst, in_=src, func=Act.Copy)
                    nc.gpsimd.memset(wh_v[:, :, F_HID:128], 1.0)

                    # f2 for this chunk: [128, H]
                    pf2 = ps_set.tile([128, H], F32, tag="set_b")
                    nc.tensor.matmul(pf2, lhsT=xT[:, cb * 128:(cb + 1) * 128],
                                     rhs=wa2, start=True, stop=True)
                    nc.vector.tensor_copy(out=f2sb[:, cb * H:(cb + 1) * H], in_=pf2)

                # f1 for the remaining heads
                for h in range(2, H):
                    emit_f1(h)

            with ExitStack() as psctx:
                ps_w2 = psctx.enter_context(
                    tc.tile_pool(name="ps_w2", bufs=1, space="PSUM"))
                ps_main = psctx.enter_context(
                    tc.tile_pool(name="ps_main", bufs=3, space="PSUM"))

                # ---------------- layer 1 ----------------
                for hp in range(H // 2):
                    ha, hb = 2 * hp, 2 * hp + 1
                    ph1s = [ps_main.tile([128, RSH], F32, tag="h1", name=f"ph1_{h}")
                            for h in (ha, hb)]
                    for cb in range(NCB):
                        site = hp * NCB + cb
                        u = work.tile([128, RSH * 2], F16, tag="u", bufs=6)
                        if SCHED[site % len(SCHED)] == "A":
                            # ACT-heavy: lrelu+outer-add fused via Prelu bias;
                            # additive mask after (Pool-free, DVE-light)
                            for i, h in enumerate((ha, hb)):
                                nc.scalar.activation(
                                    out=u[:, i * RSH:(i + 1) * RSH],
                                    in_=f1pair[hp][:, i * RSH:(i + 1) * RSH],
                                    func=Act.Prelu,
                                    bias=f2sb[:, cb * H + h:cb * H + h + 1],
                                    alpha=ALPHA)
                            mv = madj[cb].rearrange("p (r f) -> p r f", r=1) \
                                .broadcast_to((128, 2, RSH))
                            nc.vector.tensor_tensor(
                                out=u.rearrange("p (r f) -> p r f", r=2),
                                in0=u.rearrange("p (r f) -> p r f", r=2),
                                in1=mv, op=Alu.add)
                        else:
                            # u = (madj + f2) + f1  (ts-ptr 4x, tt 2x);
                            # alpha-mult on Pool (ts-imm), max on DVE
                            for i, h in enumerate((ha, hb)):
                                nc.vector.tensor_scalar(
                                    out=u[:, i * RSH:(i + 1) * RSH], in0=madj[cb],
                                    scalar1=f2sb[:, cb * H + h:cb * H + h + 1],
                                    scalar2=None, op0=Alu.add)
                            nc.vector.tensor_tensor(out=u, in0=u, in1=f1pair[hp],
                                                    op=Alu.add)
                            au = work.tile([128, RSH * 2], F16, tag="au", bufs=3)
                            nc.gpsimd.tensor_scalar(out=au, in0=u, scalar1=ALPHA,
                                                    scalar2=None, op0=Alu.mult)
                            nc.vector.tensor_tensor(out=u, in0=u, in1=au, op=Alu.max)
                        p = work.tile([128, RSH * 2], F16, tag="p", bufs=5)
                        nc.scalar.activation(out=p, in_=u, func=Act.Exp)
                        for i, h in enumerate((ha, hb)):
                            for j in range(2):
                                nc.tensor.matmul(
                                    ph1s[i][:, j * 512:(j + 1) * 512],
                                    lhsT=whaug[cb][:, h * 128:(h + 1) * 128],
                                    rhs=p[:, i * RSH + j * 512:i * RSH + (j + 1) * 512],
                                    start=(cb == 0), stop=(cb == NCB - 1))
                    # epilogue (both heads): normalize + ELU -> hT.
                    # DVE lanes are partition-fixed: recip stays on rows
                    # 64:128; a DMA shifts it down to rows 0:64.
                    # For the last head pair, split into row-halves so the
                    # Wh2 matmuls (and then the AllGather) start sooner.
                    halves = ((slice(0, RSH),) if hp < H // 2 - 1
                              else (slice(0, RSH // 2), slice(RSH // 2, RSH)))
                    for i, h in enumerate((ha, hb)):
                        ph1 = ph1s[i]
                        rs = ep.tile([128, RSH], F32, tag="rs")
                        hn = ep.tile([64, RSH], F16, tag="hn")
                        m = ep.tile([64, RSH], F16, tag="m")
                        tmp2 = ep.tile([64, RSH], F16, tag="hodd")
                        for sl in halves:
                            nc.vector.reciprocal(out=rs[64:128, sl], in_=ph1[64:128, sl])
                            nc.sync.dma_start(out=rs[0:64, sl], in_=rs[64:128, sl])
                            nc.vector.scalar_tensor_tensor(out=hn[:, sl], in0=ph1[0:64, sl],
                                                           scalar=0.0, in1=rs[0:64, sl],
                                                           op0=Alu.add, op1=Alu.mult)
                            # ELU(hn) = max(exp(min(hn,0)) - 1, hn)
                            nc.gpsimd.tensor_scalar(out=m[:, sl], in0=hn[:, sl], scalar1=0.0,
                                                    scalar2=None, op0=Alu.min)
                            nc.scalar.activation(out=m[:, sl], in_=m[:, sl], func=Act.Exp)
                            nc.gpsimd.tensor_scalar(out=m[:, sl], in0=m[:, sl], scalar1=-1.0,
                                                    scalar2=None, op0=Alu.add)
                            if h % 2 == 0:
                                nc.vector.tensor_tensor(out=hT[h // 2][0:64, sl], in0=m[:, sl],
                                                        in1=hn[:, sl], op=Alu.max)
                            else:
                                nc.vector.tensor_tensor(out=tmp2[:, sl], in0=m[:, sl],
                                                        in1=hn[:, sl], op=Alu.max)
                                nc.sync.dma_start(out=hT[h // 2][64:128, sl], in_=tmp2[:, sl])

                # ---------------- Wh2 + exchange ----------------
                ccsb = res.tile([128, 8 * 65], F16, tag="ccsb")
                for nb in range(8):
                    pw2 = ps_w2.tile([128, 65], F32, tag="w2", bufs=2)
                    for k in range(4):
                        nc.tensor.matmul(pw2, lhsT=hT[k][:, nb * 128:(nb + 1) * 128],
                                         rhs=woaug[k], start=(k == 0), stop=(k == 3))
                    nc.vector.tensor_copy(out=ccsb[:, nb * 65:(nb + 1) * 65], in_=pw2)
                # ship only the 33 real columns (32 Wh2 + g2); the 32
                # ones-columns are reinserted locally during the collective
                ccsb_v = ccsb.rearrange("p (nb j) -> p nb j", j=65)
                cc_in_v = cc_in[:, :].rearrange("(nb p) j -> p nb j", p=128)
                nc.sync.dma_start(out=cc_in_v[:, :, 0:32], in_=ccsb_v[:, :, 0:32])
                nc.sync.dma_start(out=cc_in_v[:, :, 32:33], in_=ccsb_v[:, :, 64:65])
                nc.gpsimd.collective_compute(
                    "AllGather", Alu.bypass, replica_groups=GROUPS,
                    ins=[cc_in[:, :]], outs=[cc_out[:, :]])
                wh2all = res.tile([128, NCB * 65], F16, tag="wh2all")
                wh2all_v = wh2all.rearrange("p (cb j) -> p cb j", j=65)
                nc.gpsimd.memset(wh2all_v[:, :, 32:64], 1.0)
                cc_out_v = cc_out[:, :].rearrange("(cb p) j -> p cb j", p=128)
                nc.sync.dma_start(out=wh2all_v[:, :, 0:32], in_=cc_out_v[:, :, 0:32])
                nc.sync.dma_start(out=wh2all_v[:, :, 64:65], in_=cc_out_v[:, :, 32:33])
                wh2 = [wh2all[:, cb * 65:(cb + 1) * 65] for cb in range(NCB)]
                # g2 scalars (col 64 of each block) as f32 for ts-ptr
                g2sb = res.tile([128, NCB], F32, tag="g2sb")
                nc.vector.tensor_copy(
                    out=g2sb,
                    in_=wh2all.rearrange("p (cb j) -> p cb j", j=65)[:, :, 64])

                # g1 replicated: [128, 1024]
                pg1 = ps_main.tile([128, RSH], F32, tag="h1")
                for j in range(2):
                    for k in range(4):
                        nc.tensor.matmul(pg1[:, j * 512:(j + 1) * 512],
                                         lhsT=wo1rep[k],
                                         rhs=hT[k][:, j * 512:(j + 1) * 512],
                                         start=(k == 0), stop=(k == 3))
                g1rep = res.tile([128, RSH], F16, tag="g1rep")
                nc.scalar.activation(out=g1rep, in_=pg1, func=Act.Copy)
                # Fill the AllGather window: madj[cc] += g1 in place, so the
                # post-collective per-iteration DVE work shrinks to two
                # 4x ts-adds (the g2 scalar) plus the alpha mult.
                for cc in range(NCB):
                    nc.vector.tensor_tensor(out=madj[cc], in0=madj[cc], in1=g1rep,
                                            op=Alu.add)

                # ---------------- layer 2 ----------------
                po = ps_main.tile([128, RSH], F32, tag="h1")
                for cbp in range(NCB // 2):
                    ca, cb2 = 2 * cbp, 2 * cbp + 1
                    site = 64 + cbp
                    u2 = work.tile([128, RSH * 2], F16, tag="u", bufs=6)
                    for i, cc in enumerate((ca, cb2)):
                        nc.vector.tensor_scalar(
                            out=u2[:, i * RSH:(i + 1) * RSH], in0=madj[cc],
                            scalar1=g2sb[:, cc:cc + 1],
                            scalar2=None, op0=Alu.add)
                    if SCHED[site % len(SCHED)] == "A":
                        nc.scalar.activation(out=u2, in_=u2, func=Act.Prelu,
                                             alpha=ALPHA)
                    else:
                        au = work.tile([128, RSH * 2], F16, tag="au", bufs=3)
                        nc.gpsimd.tensor_scalar(out=au, in0=u2, scalar1=ALPHA,
                                                scalar2=None, op0=Alu.mult)
                        nc.vector.tensor_tensor(out=u2, in0=u2, in1=au, op=Alu.max)
                    p2 = work.tile([128, RSH * 2], F16, tag="p", bufs=5)
                    nc.scalar.activation(out=p2, in_=u2, func=Act.Exp)
                    for i, cc in enumerate((ca, cb2)):
                        for j in range(2):
                            nc.tensor.matmul(
                                po[0:65, j * 512:(j + 1) * 512],
                                lhsT=wh2[cc],
                                rhs=p2[:, i * RSH + j * 512:i * RSH + (j + 1) * 512],
                                start=(cc == 0), stop=(cc == NCB - 1))
                rs2 = fin.tile([64, RSH], F32, tag="rs2")
                ov = fin.tile([32, RSH], F32, tag="ov")
                osb = fin.tile([32, RSH], F32, tag="osb")
                for sl in (slice(0, RSH // 2), slice(RSH // 2, RSH)):
                    nc.vector.reciprocal(out=rs2[32:64, sl], in_=po[32:64, sl])
                    nc.sync.dma_start(out=rs2[0:32, sl], in_=rs2[32:64, sl])
                    nc.vector.scalar_tensor_tensor(out=ov[:, sl], in0=po[0:32, sl], scalar=0.0,
                                                   in1=rs2[0:32, sl], op0=Alu.add, op1=Alu.mult)
                    nc.vector.scalar_tensor_tensor(out=osb[:, sl], in0=ov[:, sl], scalar=OUT_SLOPE,
                                                   in1=ov[:, sl], op0=Alu.mult, op1=Alu.max)
                    nc.sync.dma_start(out=out_e[:, sl], in_=osb[:, sl])
    from concourse.library_overlay import lower_extended_insts
    lower_extended_insts(nc)
    if legalize:
        _legalize_multi_waits(nc)
    return nc


_NC = None


def _host_prep(x, adj, W, a1, a2, Wout, ao1, ao2):
    x = np.asarray(x, dtype=np.float32)
    adj = np.asarray(adj, dtype=np.float32)
    W = np.asarray(W, dtype=np.float32)
    a1 = np.asarray(a1, dtype=np.float32)
    a2 = np.asarray(a2, dtype=np.float32)
    Wout = np.asarray(Wout, dtype=np.float32)
    ao1 = np.asarray(ao1, dtype=np.float32)
    ao2 = np.asarray(ao2, dtype=np.float32)

    xT = np.ascontiguousarray(x.transpose(0, 2, 1)).astype(np.float16)  # [B,128,N]
    madj = ((adj.transpose(0, 2, 1) - 1.0) * (-MASK)).astype(np.float16)  # 0 / -150, [B,N,N] as (c,r)
    wall = np.ascontiguousarray(W.transpose(1, 0, 2).reshape(F_IN, H * F_HID)).astype(np.float16)
    wa1 = np.einsum('hfo,ho->fh', W, a1)                           # [128,H]
    wa2 = np.ascontiguousarray(np.einsum('hfo,ho->fh', W, a2)).astype(np.float16)
    w1rep = np.repeat(wa1, 128, axis=1).astype(np.float16)         # [128,H*128]
    wo1 = Wout @ ao1                                               # [512]
    wo2 = Wout @ ao2
    woaug = np.zeros((H * F_HID, 65), np.float16)
    woaug[:, :C] = Wout.astype(np.float16)
    woaug[:, 64] = wo2.astype(np.float16)
    wo1rep = np.repeat(wo1[:, None], 128, axis=1).astype(np.float16)

    in_maps = []
    for c in range(8):
        b, s = c // 2, c % 2
        sl = slice(s * RSH, (s + 1) * RSH)
        in_maps.append({
            "xT": np.ascontiguousarray(xT[b]),
            "xtr": np.ascontiguousarray(xT[b][:, sl]),
            "madj": np.ascontiguousarray(madj[b][:, sl]),
            "wall": wall,
            "wa2": wa2,
            "w1rep": np.ascontiguousarray(w1rep),
            "woaug": woaug,
            "wo1rep": wo1rep,
        })
    return in_maps


def run(x, adj, W, a1, a2, Wout, ao1, ao2, trace=False, **trace_kw):
    global _NC
    if _NC is None:
        _NC = build_nc()
    in_maps = _host_prep(x, adj, W, a1, a2, Wout, ao1, ao2)
    r = run_bass_kernel_spmd(_NC, in_maps, list(range(8)), trace=trace, **trace_kw)
    out = np.empty((B, N, C), np.float32)
    for c in range(8):
        b, s = c // 2, c % 2
        out[b, s * RSH:(s + 1) * RSH, :] = r.results[c]["out"].T
    return out, r


def kernel(x, adj, W, a1, a2, Wout, ao1, ao2, batch_size=None):
    out, _ = run(x, adj, W, a1, a2, Wout, ao1, ao2)
    return out



# revision 17
# speedup vs baseline: 333.8687x; 333.8687x over previous
"""Dense 2-layer GAT on 8 Trainium2 NeuronCores (Bass/Tile, SPMD) — v4.

Problem: B=4 graphs, N=2048 nodes, F_in=128, H=8 heads, F_hid=64, C=32.
Sharding: 2 cores per graph, each core owns 1024 attention rows (r-shard)
for all heads in layer 1 and for layer 2; only a tiny [1024,33] AllGather
of Wh2 crosses cores between the layers.

v3 (factorized exp): HW profiling showed the v2 per-site chain
(ts -> tt -> Pool mult -> tt max -> ACT exp) pays ~1us of cross-engine
semaphore latency per hop.  v3 removed the per-site exp and Pool
round-trips:
    exp(lrelu(e)) = max(exp(e), exp(alpha*e))          (alpha > 0)
    exp(e - EB)   = exp(f1 - EB/2) * exp(f2 - EB/2)    (rank-1 factoring)
    additive -inf mask == multiplicative 0/1 adj mask after exp
Per (head-pair, chunk) site: 2 DVE ts-ptr (exp(f1)*exp(f2) scalar), 2 ACT
Exp (alpha-branch, fused per-partition bias = alpha*f2-EB), 1 DVE max,
1 DVE mask-mult.  exp biases: EB=3 for layer 1 (e in [-10,10.4], products
<= e^7.42 = 1672 < f16 max), 0 for layer 2 (e2 in [-2.4,3.3]).

v4: epilogue rework (was ~330us of the 650us/rep on HW).  The old
whaug = [Wh|ones] single matmul put the softmax denominator on PSUM
partitions 64:128, forcing a per-head SBUF DMA shift (recip is
partition-fixed on DVE) plus a Pool/ACT/DVE ELU chain per head.  v4
accumulates numerator and denominator in SEPARATE PSUM tiles at the SAME
partitions: num lhsT = Wh cols (head even -> out partitions 0:64, head
odd -> 64:128 via the out AP base), den lhsT = a memset ones [128,64].
The per-head-pair epilogue is then 5 full-width ops with no DMA:
    rs = recip(pden)                      DVE [128,1024]
    hn = pnum * rs                        DVE stt (PSUM x SBUF)
    m  = exp(hn)                          ACT (exp(min(x,0)) = min(exp x,1))
    m2 = (m min 1) + (-1)                 Pool ts (two scalar ops)
    hT[hp] = max(m2, hn)                  DVE tt   (ELU, both heads)
Layer 2 uses the same split (po_num/po_den at partitions 0:32), removing
the final shift as well; wh2/woaug lose their ones columns (33-wide
exchange blocks, single contiguous collective copies).
"""
import numpy as np
import ml_dtypes

import concourse.bass as bass
import concourse.mybir as mybir
import concourse.tile as tile
from concourse.bass_utils import run_bass_kernel_spmd
from concourse.vector_clock import VectorClock, ScopedClock

F32 = mybir.dt.float32
F16 = mybir.dt.float16
F8 = mybir.dt.float8e4
Alu = mybir.AluOpType
Act = mybir.ActivationFunctionType

B, N, F_IN, H, F_HID, C = 4, 2048, 128, 8, 64, 32
RSH = N // 2           # rows per core
NCB = N // 128         # 16 c-chunks
ALPHA = 0.2
OUT_SLOPE = 0.01
EB = 3.0               # layer-1 exp bias (softmax-invariant)
GROUPS = [[0, 1], [2, 3], [4, 5], [6, 7]]

# ---------------------------------------------------------------------------
# Patch: Tile's kernel-tail drain aggregates one wait per outstanding proc
# lane into a single Drain instruction; walrus codegen only supports one
# sync wait per instruction ("Too many sync wait commands").  Split into a
# chain of single-wait drains instead.
import concourse.tile as _tile_mod

_ORIG_DRAIN = _tile_mod.TileContext._drain_and_barrier


def _split_drain_and_barrier(self, tick_clock, wait_clock):
    vals = eval(repr(tick_clock.global_clock).split("VectorClock(", 1)[1].rstrip(")"))
    for i, v in enumerate(vals):
        if v <= 0:
            continue
        part = VectorClock()
        part.require_at_least(i, v)
        d = self.nc.sync.drain()
        wait_clock.add_sem_waits(d.ins, ScopedClock({None: part}))
    self.nc.sync.drain()
    self.nc.all_engine_barrier()
    popped = self.nc._tile_sem_poison_stack.pop()
    assert popped is self._sem_poison
    self.nc.clear_and_free_semaphores(list(self.sems.allocated().values()))
    self.nc.all_engine_barrier()


_tile_mod.TileContext._drain_and_barrier = _split_drain_and_barrier


def _legalize_multi_waits(nc):
    """Walrus codegen accepts at most one sync wait per instruction; hoist
    extra waits onto preceding same-engine sequencer NOPs."""
    Op = nc.isa.Opcode

    def mk_nop(engine):
        return nc.engines[engine]._isa(Op.NEURON_ISA_TPB_OPCODE_NOP, {})

    n_fix = 0
    for f in nc.m.functions:
        for bb in f.blocks:
            insts = list(bb.instructions)
            if not any(i.sync_info and i.sync_info.on_wait
                       and len(i.sync_info.on_wait) > 1 for i in insts):
                continue
            new = []
            for inst in insts:
                si = inst.sync_info
                if si and si.on_wait and len(si.on_wait) > 1:
                    waits = list(si.on_wait)
                    for w in waits[:-1]:
                        nop = mk_nop(inst.engine)
                        nop.sync_info = mybir.SyncInfo(on_wait=[w], on_update=[])
                        new.append(nop)
                        n_fix += 1
                    inst.sync_info = mybir.SyncInfo(
                        on_wait=[waits[-1]], on_update=list(si.on_update or []))
                new.append(inst)
            bb.instructions = new
    return n_fix
# ---------------------------------------------------------------------------


def build_nc(legalize=True, collective=True, reps=1):
    nc = bass.Bass(num_devices=8)

    xT_e = nc.dram_tensor("xT", [F_IN, N], F16, kind="ExternalInput")
    xtr_e = nc.dram_tensor("xtr", [F_IN, RSH], F16, kind="ExternalInput")
    adj_e = nc.dram_tensor("adj01", [N, RSH], F16, kind="ExternalInput")
    wall_e = nc.dram_tensor("wall", [F_IN, H * F_HID], F16, kind="ExternalInput")
    wa2_e = nc.dram_tensor("wa2", [F_IN, H], F16, kind="ExternalInput")
    w1rep_e = nc.dram_tensor("w1rep", [F_IN, H * 128], F16, kind="ExternalInput")
    woaug_e = nc.dram_tensor("woaug", [H * F_HID, 33], F16, kind="ExternalInput")
    wo1rep_e = nc.dram_tensor("wo1rep", [H * F_HID, 128], F16, kind="ExternalInput")
    out_e = nc.dram_tensor("out", [C, RSH], F32, kind="ExternalOutput")
    cc_ins = [nc.dram_tensor(f"cc_in{i}", [RSH, 33], F16) for i in range(min(reps, 2))]
    cc_outs = [nc.dram_tensor(f"cc_out{i}", [N, 33], F16) for i in range(min(reps, 2))]

    with tile.TileContext(nc) as tc:
        from contextlib import ExitStack
        with ExitStack() as ctx:
            res = ctx.enter_context(tc.tile_pool(name="res", bufs=1))
            adjp = ctx.enter_context(tc.tile_pool(name="adjp", bufs=2))
            work = ctx.enter_context(tc.tile_pool(name="work", bufs=4))
            ep = ctx.enter_context(tc.tile_pool(name="ep", bufs=2))
            fin = ctx.enter_context(tc.tile_pool(name="fin", bufs=1))

            ebc = res.tile([128, 1], F32, tag="ebc")
            nc.gpsimd.memset(ebc, -EB / 2)
            ebf = res.tile([128, 1], F32, tag="ebf")
            nc.gpsimd.memset(ebf, -EB)
            ones64 = res.tile([128, 64], F16, tag="ones64")
            nc.gpsimd.memset(ones64, 1.0)

            for rep in range(reps):
                cc_in = cc_ins[rep % len(cc_ins)]
                cc_out = cc_outs[rep % len(cc_outs)]

                # ---------------- input loads ----------------
                # order: what the first Wh matmul + first L1 site need first
                wall = res.tile([F_IN, H * F_HID], F16, tag="wall")
                nc.sync.dma_start(out=wall, in_=wall_e[:, :])
                xT = res.tile([F_IN, N], F16, tag="xT")
                nc.sync.dma_start(out=xT[:, 0:512], in_=xT_e[:, 0:512])
                wa2 = res.tile([F_IN, H], F16, tag="wa2")
                nc.sync.dma_start(out=wa2, in_=wa2_e[:, :])
                xtr = res.tile([F_IN, RSH], F16, tag="xtr")
                nc.sync.dma_start(out=xtr, in_=xtr_e[:, :])
                w1rep = res.tile([F_IN, H * 128], F16, tag="w1rep")
                nc.sync.dma_start(out=w1rep, in_=w1rep_e[:, :])
                # adjacency as one [128, NCB*RSH] mega-tile (chunk cb at cols
                # cb*RSH:(cb+1)*RSH) so layer-2 pair-sites get contiguous
                # masks.  Split the 4MB load across both HWDGE queues; the
                # adjp pool is double-buffered so rep i+1's reload overlaps
                # rep i's layer-2 reads.
                adj01 = adjp.tile([128, NCB * RSH], F16, tag="adj01")
                adj_v = adj_e[:, :].rearrange("(cb p) r -> p cb r", p=128)
                adj01_v = adj01.rearrange("p (cb r) -> p cb r", r=RSH)
                for cb in range(NCB):
                    eng = nc.sync if cb % 2 == 0 else nc.scalar
                    eng.dma_start(out=adj01_v[:, cb, :], in_=adj_v[:, cb, :])
                for j in range(1, 4):
                    nc.sync.dma_start(out=xT[:, j * 512:(j + 1) * 512],
                                      in_=xT_e[:, j * 512:(j + 1) * 512])
                woaug = [res.tile([128, 33], F16, tag=f"woaug{k}", name=f"woaug{k}_r{rep}") for k in range(4)]
                wo1rep = [res.tile([128, 128], F16, tag=f"wo1rep{k}", name=f"wo1rep{k}_r{rep}") for k in range(4)]
                for k in range(4):
                    nc.sync.dma_start(out=woaug[k], in_=woaug_e[k * 128:(k + 1) * 128, :])
                    nc.sync.dma_start(out=wo1rep[k], in_=wo1rep_e[k * 128:(k + 1) * 128, :])

                # Wh only (no ones columns): head h at cols h*64:(h+1)*64
                wh = [res.tile([128, H * F_HID], F16, tag=f"wh{cb}", name=f"wh{cb}_r{rep}") for cb in range(NCB)]
                # f1 for a head PAIR side by side: [128, 2*RSH]; raw (for the
                # ACT alpha-branch) and exp'd (for the DVE one-branch)
                f1pair = [res.tile([128, 2 * RSH], F16, tag=f"f1pair{hp}", name=f"f1pair{hp}_r{rep}")
                          for hp in range(H // 2)]
                ef1pair = [res.tile([128, 2 * RSH], F16, tag=f"ef1pair{hp}", name=f"ef1pair{hp}_r{rep}")
                           for hp in range(H // 2)]
                f2sb = res.tile([128, NCB * H], F32, tag="f2sb")
                ef2sb = res.tile([128, NCB * H], F32, tag="ef2sb")
                ba_sb = res.tile([128, NCB * H], F32, tag="ba_sb")
                hT = [res.tile([128, RSH], F16, tag=f"hT{k}", name=f"hT{k}_r{rep}") for k in range(4)]

                with tc.tile_pool(name="ps_set", bufs=2, space="PSUM") as ps_set:
                    # f1 for the first head pair before anything else: the
                    # first L1 site needs f1pair[0]/ef1pair[0]; the Wh chunk
                    # loop is long.
                    def emit_f1(h):
                        pf1 = ps_set.tile([128, RSH], F32, tag="set_f1")
                        for j in range(2):
                            nc.tensor.matmul(pf1[:, j * 512:(j + 1) * 512],
                                             lhsT=w1rep[:, h * 128:(h + 1) * 128],
                                             rhs=xtr[:, j * 512:(j + 1) * 512],
                                             start=True, stop=True)
                        hp, i = h // 2, h % 2
                        sl = slice(i * RSH, (i + 1) * RSH)
                        if h >= 2 and h % 2 == 0:
                            nc.vector.tensor_copy(out=f1pair[hp][:, sl], in_=pf1)
                        else:
                            nc.scalar.activation(out=f1pair[hp][:, sl], in_=pf1,
                                                 func=Act.Copy)
                        # exp(f1 - EB/2) straight from PSUM on ACT
                        nc.scalar.activation(out=ef1pair[hp][:, sl], in_=pf1,
                                             func=Act.Exp, bias=ebc[:, 0:1])
                    for h in (0, 1):
                        emit_f1(h)
                    # Wh per c-chunk: [128, 512] = all heads side by side
                    for cb in range(NCB):
                        pwh = ps_set.tile([128, H * F_HID], F32, tag="set_a")
                        nc.tensor.matmul(pwh, lhsT=xT[:, cb * 128:(cb + 1) * 128],
                                         rhs=wall, start=True, stop=True)
                        if cb % 2 == 0:
                            nc.vector.tensor_copy(out=wh[cb], in_=pwh)
                        else:
                            nc.scalar.activation(out=wh[cb], in_=pwh, func=Act.Copy)

                        # f2 for this chunk: [128, H]
                        pf2 = ps_set.tile([128, H], F32, tag="set_b")
                        nc.tensor.matmul(pf2, lhsT=xT[:, cb * 128:(cb + 1) * 128],
                                         rhs=wa2, start=True, stop=True)
                        nc.vector.tensor_copy(out=f2sb[:, cb * H:(cb + 1) * H], in_=pf2)
                        # per-chunk f2 tables so the first sites need not
                        # wait for the whole f2sb: exp(f2-EB/2), alpha*f2-EB
                        csl = slice(cb * H, (cb + 1) * H)
                        nc.scalar.activation(out=ef2sb[:, csl], in_=f2sb[:, csl],
                                             func=Act.Exp, bias=ebc[:, 0:1])
                        nc.vector.tensor_scalar(out=ba_sb[:, csl], in0=f2sb[:, csl],
                                                scalar1=ALPHA, scalar2=-EB,
                                                op0=Alu.mult, op1=Alu.add)

                    # f1 for the remaining heads
                    for h in range(2, H):
                        emit_f1(h)

                with ExitStack() as psctx:
                    ps_w2 = psctx.enter_context(
                        tc.tile_pool(name="ps_w2", bufs=1, space="PSUM"))
                    ps_main = psctx.enter_context(
                        tc.tile_pool(name="ps_main", bufs=3, space="PSUM"))

                    # ---------------- layer 1 ----------------
                    # deferred epilogue tails: the final ELU max (DVE) of head
                    # pair hp is emitted inside hp+1's site loop so the DVE
                    # stream never stalls on the ACT->Pool ELU chain
                    pending_tail = []
                    for hp in range(H // 2):
                        ha, hb = 2 * hp, 2 * hp + 1
                        # numerators (head even -> partitions 0:64, odd ->
                        # 64:128) and denominators, separate banks, same
                        # partitions
                        pnum = ps_main.tile([128, RSH], F32, tag="h1",
                                            name=f"pnum{hp}_r{rep}")
                        pden = ps_main.tile([128, RSH], F32, tag="h1",
                                            name=f"pden{hp}_r{rep}")
                        for cb in range(NCB):
                            site = hp * NCB + cb
                            qa = work.tile([128, RSH * 2], F16, tag="qa", bufs=4)
                            q = work.tile([128, RSH * 2], F16, tag="q", bufs=4)
                            if site % 6 == 3:
                                # A-site: lrelu+exp fully on ACT, no DVE max
                                # t = Prelu(f1 + f2); q = exp(t - EB)
                                for i, h in enumerate((ha, hb)):
                                    nc.scalar.activation(
                                        out=qa[:, i * RSH:(i + 1) * RSH],
                                        in_=f1pair[hp][:, i * RSH:(i + 1) * RSH],
                                        func=Act.Prelu, alpha=ALPHA,
                                        bias=f2sb[:, cb * H + h:cb * H + h + 1])
                                nc.scalar.activation(out=q, in_=qa, func=Act.Exp,
                                                     bias=ebf[:, 0:1])
                            else:
                                # D-site: alpha-branch on ACT, one-branch on
                                # DVE, max on DVE
                                for i, h in enumerate((ha, hb)):
                                    nc.scalar.activation(
                                        out=qa[:, i * RSH:(i + 1) * RSH],
                                        in_=f1pair[hp][:, i * RSH:(i + 1) * RSH],
                                        func=Act.Exp, scale=ALPHA,
                                        bias=ba_sb[:, cb * H + h:cb * H + h + 1])
                                for i, h in enumerate((ha, hb)):
                                    nc.vector.tensor_scalar(
                                        out=q[:, i * RSH:(i + 1) * RSH],
                                        in0=ef1pair[hp][:, i * RSH:(i + 1) * RSH],
                                        scalar1=ef2sb[:, cb * H + h:cb * H + h + 1],
                                        scalar2=None, op0=Alu.mult)
                                # p = max(q, qa) * adj01  (DVE only)
                                nc.vector.tensor_tensor(out=q, in0=q, in1=qa, op=Alu.max)
                            if cb == 2 and pending_tail:
                                for fn in pending_tail:
                                    fn()
                                pending_tail = []
                            mv = adj01_v[:, cb, :].rearrange("p (r f) -> p r f", r=1) \
                                .broadcast_to((128, 2, RSH))
                            nc.vector.tensor_tensor(
                                out=q.rearrange("p (r f) -> p r f", r=2),
                                in0=q.rearrange("p (r f) -> p r f", r=2),
                                in1=mv, op=Alu.mult)
                            for i, h in enumerate((ha, hb)):
                                po = slice(i * 64, (i + 1) * 64)
                                for j in range(2):
                                    rhs = q[:, i * RSH + j * 512:i * RSH + (j + 1) * 512]
                                    cs = slice(j * 512, (j + 1) * 512)
                                    nc.tensor.matmul(
                                        pnum[po, cs],
                                        lhsT=wh[cb][:, h * F_HID:(h + 1) * F_HID],
                                        rhs=rhs,
                                        start=(cb == 0), stop=(cb == NCB - 1))
                                    nc.tensor.matmul(
                                        pden[po, cs], lhsT=ones64, rhs=rhs,
                                        start=(cb == 0), stop=(cb == NCB - 1))
                        # epilogue (both heads at once, no partition shift):
                        # hT[hp] = ELU(pnum/pden); ELU(x) = max(min(exp(x),1)-1, x)
                        halves = ((slice(0, RSH),) if hp < H // 2 - 1
                                  else (slice(0, RSH // 2), slice(RSH // 2, RSH)))
                        rs = ep.tile([128, RSH], F32, tag="rs")
                        hn = ep.tile([128, RSH], F16, tag="hn")
                        m = ep.tile([128, RSH], F16, tag="m")
                        for sl in halves:
                            nc.vector.reciprocal(out=rs[:, sl], in_=pden[:, sl])
                            nc.vector.scalar_tensor_tensor(
                                out=hn[:, sl], in0=pnum[:, sl], scalar=0.0,
                                in1=rs[:, sl], op0=Alu.add, op1=Alu.mult)
                            nc.scalar.activation(out=m[:, sl], in_=hn[:, sl],
                                                 func=Act.Exp)
                            nc.gpsimd.tensor_scalar(out=m[:, sl], in0=m[:, sl],
                                                    scalar1=1.0, scalar2=-1.0,
                                                    op0=Alu.min, op1=Alu.add)
                            if hp < H // 2 - 1:
                                pending_tail.append(
                                    lambda hp=hp, sl=sl, m=m, hn=hn:
                                    nc.vector.tensor_tensor(
                                        out=hT[hp][:, sl], in0=m[:, sl],
                                        in1=hn[:, sl], op=Alu.max))
                            else:
                                nc.vector.tensor_tensor(out=hT[hp][:, sl], in0=m[:, sl],
                                                        in1=hn[:, sl], op=Alu.max)

                    # ---------------- Wh2 + exchange ----------------
                    ccsb = res.tile([128, 8 * 33], F16, tag="ccsb")
                    for nb in range(8):
                        pw2 = ps_w2.tile([128, 33], F32, tag="w2", bufs=2)
                        for k in range(4):
                            nc.tensor.matmul(pw2, lhsT=hT[k][:, nb * 128:(nb + 1) * 128],
                                             rhs=woaug[k], start=(k == 0), stop=(k == 3))
                        nc.vector.tensor_copy(out=ccsb[:, nb * 33:(nb + 1) * 33], in_=pw2)
                    cc_in_v = cc_in[:, :].rearrange("(nb p) j -> p nb j", p=128)
                    nc.sync.dma_start(
                        out=cc_in_v,
                        in_=ccsb.rearrange("p (nb j) -> p nb j", j=33))
                    if collective:
                        nc.gpsimd.collective_compute(
                            "AllGather", Alu.bypass, replica_groups=GROUPS,
                            ins=[cc_in[:, :]], outs=[cc_out[:, :]])
                    else:
                        # timing-only ablation: local copy instead of AllGather
                        nc.sync.dma_start(out=cc_out[0:RSH, :], in_=cc_in[:, :])
                        nc.sync.dma_start(out=cc_out[RSH:N, :], in_=cc_in[:, :])
                    wh2all = res.tile([128, NCB * 33], F16, tag="wh2all")
                    nc.sync.dma_start(
                        out=wh2all.rearrange("p (cb j) -> p cb j", j=33),
                        in_=cc_out[:, :].rearrange("(cb p) j -> p cb j", p=128))
                    wh2 = [wh2all[:, cb * 33:cb * 33 + 32] for cb in range(NCB)]
                    # g2 tables (col 32 of each block): exp(g2) and alpha*g2
                    g2sb = res.tile([128, NCB], F32, tag="g2sb")
                    nc.vector.tensor_copy(
                        out=g2sb,
                        in_=wh2all.rearrange("p (cb j) -> p cb j", j=33)[:, :, 32])
                    eg2sb = res.tile([128, NCB], F32, tag="eg2sb")
                    nc.scalar.activation(out=eg2sb, in_=g2sb, func=Act.Exp)
                    bg2sb = res.tile([128, NCB], F32, tag="bg2sb")
                    nc.vector.tensor_scalar(out=bg2sb, in0=g2sb, scalar1=ALPHA,
                                            scalar2=None, op0=Alu.mult)

                    # g1 replicated: [128, 1024]; raw + exp'd tables
                    pg1 = ps_main.tile([128, RSH], F32, tag="h1")
                    for j in range(2):
                        for k in range(4):
                            nc.tensor.matmul(pg1[:, j * 512:(j + 1) * 512],
                                             lhsT=wo1rep[k],
                                             rhs=hT[k][:, j * 512:(j + 1) * 512],
                                             start=(k == 0), stop=(k == 3))
                    g1rep = res.tile([128, RSH], F16, tag="g1rep")
                    nc.scalar.activation(out=g1rep, in_=pg1, func=Act.Copy)
                    eg1rep = res.tile([128, RSH], F16, tag="eg1rep")
                    nc.scalar.activation(out=eg1rep, in_=pg1, func=Act.Exp)

                    # ---------------- layer 2 ----------------
                    # full-shape tiles from the rotating h1 tag (PSUM budget);
                    # only partitions 0:32 are used
                    pon = ps_main.tile([128, RSH], F32, tag="h1")
                    pod = ps_main.tile([128, RSH], F32, tag="h1")
                    for cbp in range(NCB // 2):
                        ca, cb2 = 2 * cbp, 2 * cbp + 1
                        qa2 = work.tile([128, RSH * 2], F16, tag="qa", bufs=4)
                        for i, cc in enumerate((ca, cb2)):
                            nc.scalar.activation(
                                out=qa2[:, i * RSH:(i + 1) * RSH], in_=g1rep,
                                func=Act.Exp, scale=ALPHA,
                                bias=bg2sb[:, cc:cc + 1])
                        q2 = work.tile([128, RSH * 2], F16, tag="q", bufs=4)
                        for i, cc in enumerate((ca, cb2)):
                            nc.vector.tensor_scalar(
                                out=q2[:, i * RSH:(i + 1) * RSH], in0=eg1rep,
                                scalar1=eg2sb[:, cc:cc + 1],
                                scalar2=None, op0=Alu.mult)
                        nc.vector.tensor_tensor(out=q2, in0=q2, in1=qa2, op=Alu.max)
                        nc.vector.tensor_tensor(
                            out=q2, in0=q2,
                            in1=adj01[:, ca * RSH:(ca + 2) * RSH], op=Alu.mult)
                        for i, cc in enumerate((ca, cb2)):
                            for j in range(2):
                                rhs = q2[:, i * RSH + j * 512:i * RSH + (j + 1) * 512]
                                cs = slice(j * 512, (j + 1) * 512)
                                nc.tensor.matmul(pon[0:32, cs], lhsT=wh2[cc], rhs=rhs,
                                                 start=(cc == 0), stop=(cc == NCB - 1))
                                nc.tensor.matmul(pod[0:32, cs], lhsT=ones64[:, 0:32],
                                                 rhs=rhs,
                                                 start=(cc == 0), stop=(cc == NCB - 1))
                    rs2 = fin.tile([32, RSH], F32, tag="rs2")
                    ov = fin.tile([32, RSH], F32, tag="ov")
                    osb = fin.tile([32, RSH], F32, tag="osb")
                    for sl in (slice(0, RSH // 2), slice(RSH // 2, RSH)):
                        nc.vector.reciprocal(out=rs2[:, sl], in_=pod[0:32, sl])
                        nc.vector.scalar_tensor_tensor(out=ov[:, sl], in0=pon[0:32, sl], scalar=0.0,
                                                       in1=rs2[:, sl], op0=Alu.add, op1=Alu.mult)
                        nc.vector.scalar_tensor_tensor(out=osb[:, sl], in0=ov[:, sl], scalar=OUT_SLOPE,
                                                       in1=ov[:, sl], op0=Alu.mult, op1=Alu.max)
                        nc.sync.dma_start(out=out_e[:, sl], in_=osb[:, sl])
    from concourse.library_overlay import lower_extended_insts
    lower_extended_insts(nc)
    if legalize:
        _legalize_multi_waits(nc)
    return nc


_NC = None


def _host_prep(x, adj, W, a1, a2, Wout, ao1, ao2):
    x = np.asarray(x, dtype=np.float32)
    adj = np.asarray(adj, dtype=np.float32)
    W = np.asarray(W, dtype=np.float32)
    a1 = np.asarray(a1, dtype=np.float32)
    a2 = np.asarray(a2, dtype=np.float32)
    Wout = np.asarray(Wout, dtype=np.float32)
    ao1 = np.asarray(ao1, dtype=np.float32)
    ao2 = np.asarray(ao2, dtype=np.float32)

    xT = np.ascontiguousarray(x.transpose(0, 2, 1)).astype(np.float16)  # [B,128,N]
    adj01 = adj.transpose(0, 2, 1).astype(np.float16)  # 0/1, [B,N,N] as (c,r)
    wall = np.ascontiguousarray(W.transpose(1, 0, 2).reshape(F_IN, H * F_HID)).astype(np.float16)
    wa1 = np.einsum('hfo,ho->fh', W, a1)                           # [128,H]
    wa2 = np.ascontiguousarray(np.einsum('hfo,ho->fh', W, a2)).astype(np.float16)
    w1rep = np.repeat(wa1, 128, axis=1).astype(np.float16)         # [128,H*128]
    wo2 = Wout @ ao2
    woaug = np.zeros((H * F_HID, 33), np.float16)
    woaug[:, :C] = Wout.astype(np.float16)
    woaug[:, 32] = wo2.astype(np.float16)
    wo1 = Wout @ ao1                                               # [512]
    wo1rep = np.repeat(wo1[:, None], 128, axis=1).astype(np.float16)

    in_maps = []
    for c in range(8):
        b, s = c // 2, c % 2
        sl = slice(s * RSH, (s + 1) * RSH)
        in_maps.append({
            "xT": np.ascontiguousarray(xT[b]),
            "xtr": np.ascontiguousarray(xT[b][:, sl]),
            "adj01": np.ascontiguousarray(adj01[b][:, sl]),
            "wall": wall,
            "wa2": wa2,
            "w1rep": np.ascontiguousarray(w1rep),
            "woaug": woaug,
            "wo1rep": wo1rep,
        })
    return in_maps


def run(x, adj, W, a1, a2, Wout, ao1, ao2, trace=False, **trace_kw):
    global _NC
    if _NC is None:
        _NC = build_nc()
    in_maps = _host_prep(x, adj, W, a1, a2, Wout, ao1, ao2)
    r = run_bass_kernel_spmd(_NC, in_maps, list(range(8)), trace=trace, **trace_kw)
    out = np.empty((B, N, C), np.float32)
    for c in range(8):
        b, s = c // 2, c % 2
        out[b, s * RSH:(s + 1) * RSH, :] = r.results[c]["out"].T
    return out, r


def kernel(x, adj, W, a1, a2, Wout, ao1, ao2, batch_size=None):
    out, _ = run(x, adj, W, a1, a2, Wout, ao1, ao2)
    return out
